# revision 1
# baseline (speedup 1.0000x reference)
"""Trainium2 Bass kernel for nn_Att_patch_net (vq_codebook).

Math (per batch b):
  y[d,pix]   = relu(BN(conv1x1(x)))                    -- folded into Wc, bias_d
  dist[k,pix]= ||y[:,pix]-c[k]||^2 ; A = softmax_k(scale_k*dist)   (per PIXEL)
  R[pix]     = A[pix,k]*(y[d,pix]-c[k,d])  in R^{K x D}
  E[p]       = sum_{pix in patch p} R[pix]             -- 5x5 box sums (100 patches)
  v          = sum_p E[p]/||E[p]||  ;  logits = v @ cls_w.T + cls_b

Device strategy (8 cores, 4 batches each):
  * A[pix] depends only on the pixel -> compute per-pixel softmax via matmuls.
  * ||E[p]||^2 = sum_{pix,pix' in p} <R_pix,R_pix'> with the Gram matrix
      G = YY*S1 - sum_k A A' (xc+xc'-c2)   (all small matmuls, no R materialized)
  * v = sum_pix w[pix]*R[pix] with w = M^T (1/||E||)   (transposed box filter)
  * classifier: class-sharded (125/core) over an AllGather of v.
"""
import numpy as np
from contextlib import ExitStack

B, CIN, HH, WW = 32, 2048, 14, 14
D, K, NCLASS = 128, 32, 1000
WIN = 5
PH = PW = 10
NP_ = 100          # patches
NPIX = HH * WW     # 196
PPIX = 256         # padded pixels per batch slot
NCORES = 8
BL = B // NCORES   # 4 local batches
CSH = NCLASS // NCORES  # 125 classes per core
NCH = CIN // 128   # 16 contraction chunks
BN_EPS = 1e-5

_NC_CACHE = {}
F32R_CONV = False   # conv matmuls in float32r (fast) vs float32 (exact)
F32R_GRAM = False   # gram matmuls in float32r
STAGE = 4           # 1=conv+softmax, 2=+gram/norms, 3=+v (no collective), 4=full
WRITE_V = False     # add a 'vout' external output with v^T (for 2-launch fallback)


# ----------------------------------------------------------------- device IR
def build_nc(n_rep: int = 1):
    import concourse.bass as bass
    import concourse.tile as tile
    from concourse import bacc, mybir

    f32 = mybir.dt.float32
    f32r = mybir.dt.float32r
    AF = mybir.ActivationFunctionType
    OP = mybir.AluOpType
    AX = mybir.AxisListType

    nc = bacc.Bacc("TRN2", target_bir_lowering=False, debug=False, num_devices=NCORES)

    def din(name, shape):
        return nc.dram_tensor(name, shape, f32, kind="ExternalInput").ap()

    xin = din("xin", [NCH, 128, BL * PPIX])
    wcT = din("wcT", [NCH, 128, 128])
    biasd = din("biasd", [128, 1])
    cw2T = din("cw2T", [128, K])
    scbc = din("scbc", [128, K])
    ones32 = din("ones32", [128, K])
    ebias = din("ebias", [128, 1])
    negis = din("negis", [128, 1])
    negc2 = din("negc2", [128, 1])
    ctile = din("ctile", [128, 128])
    bmT = din("bmT", [128, BL])
    bm = din("bm", [BL, 128])
    onec = din("onec", [128, 1])
    mmat = din("mmat", [NP_, PPIX])
    mtc = din("mtc", [2, 128, NP_])
    mth = din("mth", [2, 128, NP_])
    ident = din("ident", [128, 128])
    clsw = din("clsw", [K, 128, CSH])
    clsb = din("clsb", [CSH, 1])
    logits = nc.dram_tensor("logits", [CSH, B], f32, kind="ExternalOutput").ap()
    vout = (nc.dram_tensor("vout", [128, 128], f32, kind="ExternalOutput").ap()
            if WRITE_V else None)

    with tile.TileContext(nc) as tc, ExitStack() as ctx:
        cp = ctx.enter_context(tc.tile_pool(name="consts", bufs=1))
        xp = ctx.enter_context(tc.tile_pool(name="xp", bufs=3))
        yp = ctx.enter_context(tc.tile_pool(name="yp", bufs=2))
        ap_ = ctx.enter_context(tc.tile_pool(name="ap", bufs=2))
        sp = ctx.enter_context(tc.tile_pool(name="sp", bufs=2))
        gp = ctx.enter_context(tc.tile_pool(name="gp", bufs=2))
        ytp = ctx.enter_context(tc.tile_pool(name="ytp", bufs=10))
        ps = ctx.enter_context(tc.tile_pool(name="ps", bufs=1, space="PSUM"))
        dp = ctx.enter_context(tc.tile_pool(name="dp", bufs=2, space="DRAM"))

        # ---- constants (loaded once) ----
        wcT_sb = cp.tile([128, NCH, 128], f32)
        nc.gpsimd.dma_start(out=wcT_sb[:], in_=wcT.rearrange("n p m -> p n m"))
        biasd_sb = cp.tile([128, 1], f32)
        nc.gpsimd.dma_start(out=biasd_sb[:], in_=biasd[:])
        cw2T_sb = cp.tile([128, K], f32)
        nc.gpsimd.dma_start(out=cw2T_sb[:], in_=cw2T[:])
        scbc_sb = cp.tile([128, K], f32)
        nc.gpsimd.dma_start(out=scbc_sb[:], in_=scbc[:])
        ones32_sb = cp.tile([128, K], f32)
        nc.gpsimd.dma_start(out=ones32_sb[:], in_=ones32[:])
        ebias_sb = cp.tile([128, 1], f32)
        nc.gpsimd.dma_start(out=ebias_sb[:], in_=ebias[:])
        negis_sb = cp.tile([128, 1], f32)
        nc.gpsimd.dma_start(out=negis_sb[:], in_=negis[:])
        negc2_sb = cp.tile([128, 1], f32)
        nc.gpsimd.dma_start(out=negc2_sb[:], in_=negc2[:])
        ctile_sb = cp.tile([128, 128], f32)
        nc.gpsimd.dma_start(out=ctile_sb[:], in_=ctile[:])
        bmT_sb = cp.tile([128, BL], f32)
        nc.gpsimd.dma_start(out=bmT_sb[:], in_=bmT[:])
        bm_sb = cp.tile([BL, 128], f32)
        nc.gpsimd.dma_start(out=bm_sb[:], in_=bm[:])
        onec_sb = cp.tile([128, 1], f32)
        nc.gpsimd.dma_start(out=onec_sb[:], in_=onec[:])
        mmat_sb = cp.tile([NP_, PPIX], f32)
        nc.gpsimd.dma_start(out=mmat_sb[:], in_=mmat[:])
        mt_sb = cp.tile([128, 2, NP_], f32)
        nc.gpsimd.dma_start(out=mt_sb[:], in_=mtc.rearrange("q p n -> p q n"))
        mth_sb = cp.tile([128, 2, NP_], f32)
        nc.gpsimd.dma_start(out=mth_sb[:], in_=mth.rearrange("q p n -> p q n"))
        ident_sb = cp.tile([128, 128], f32)
        nc.gpsimd.dma_start(out=ident_sb[:], in_=ident[:])
        clsw_sb = cp.tile([128, K, CSH], f32)
        nc.gpsimd.dma_start(out=clsw_sb[:], in_=clsw.rearrange("k p n -> p k n"))
        clsb_sb = cp.tile([CSH, 1], f32)
        nc.gpsimd.dma_start(out=clsb_sb[:], in_=clsb[:])

        def r32(apx):
            return apx.bitcast(f32r)

        rc = r32 if F32R_CONV else (lambda a: a)
        rg = r32 if F32R_GRAM else (lambda a: a)

        for _rep in range(n_rep):
            # ================= conv + BN + ReLU =================
            pc0 = ps.tile([128, 512], f32, tag="A", bufs=2, name=f"pc0_{_rep}")
            pc1 = ps.tile([128, 512], f32, tag="A", bufs=2, name=f"pc1_{_rep}")
            for k in range(NCH):
                xt = xp.tile([128, BL * PPIX], f32, tag="xch", name=f"xt_{_rep}_{k}")
                nc.gpsimd.dma_start(out=xt[:], in_=xin[k])
                nc.tensor.matmul(pc0[:], rc(wcT_sb[:, k, :]), rc(xt[:, 0:512]),
                                 start=(k == 0), stop=(k == NCH - 1))
                nc.tensor.matmul(pc1[:], rc(wcT_sb[:, k, :]), rc(xt[:, 512:1024]),
                                 start=(k == 0), stop=(k == NCH - 1))
            y_all = yp.tile([128, BL * PPIX], f32, tag="y", name=f"y_{_rep}")
            nc.scalar.activation(y_all[:, 0:512], pc0[:], AF.Relu, bias=biasd_sb[:, 0:1])
            nc.scalar.activation(y_all[:, 512:1024], pc1[:], AF.Relu, bias=biasd_sb[:, 0:1])
            ysq = yp.tile([128, BL * PPIX], f32, tag="ysq", name=f"ysq_{_rep}")
            nc.scalar.activation(ysq[:], y_all[:], AF.Square)

            def yb(b, lo, hi):
                return y_all[:, PPIX * b + lo: PPIX * b + hi]

            # ================= per-pixel softmax ===============
            z_ps = ps.tile([128, 512], f32, tag="B", bufs=2, name=f"z_{_rep}")
            x2_ps = ps.tile([128, 512], f32, tag="B", bufs=2, name=f"x2_{_rep}")
            for b in range(BL):
                zb = z_ps[32 * b:32 * b + 32, 0:NPIX]
                nc.tensor.matmul(zb, cw2T_sb[:], yb(b, 0, NPIX), start=True, stop=False,
                                 tile_position=(0, 32 * b))
                nc.tensor.matmul(zb, scbc_sb[:], ysq[:, PPIX * b:PPIX * b + NPIX],
                                 start=False, stop=True, tile_position=(0, 32 * b))
                nc.tensor.matmul(x2_ps[32 * b:32 * b + 32, 0:NPIX], ones32_sb[:],
                                 ysq[:, PPIX * b:PPIX * b + NPIX], start=True, stop=True,
                                 tile_position=(0, 32 * b))
            a_u = ap_.tile([128, PPIX], f32, tag="au", name=f"au_{_rep}")
            nc.scalar.activation(a_u[:, 0:NPIX], z_ps[:, 0:NPIX], AF.Exp, bias=ebias_sb[:, 0:1])
            nc.vector.memset(a_u[:, NPIX:PPIX], 0.0)
            # t1 = -z/scale - c2 ; u = x2 + t1 = x2 - dist
            t1 = sp.tile([128, NPIX], f32, tag="t1", name=f"t1_{_rep}")
            nc.scalar.activation(t1[:], z_ps[:, 0:NPIX], AF.Identity,
                                 bias=negc2_sb[:, 0:1], scale=negis_sb[:, 0:1])
            u_sb = sp.tile([128, NPIX], f32, tag="u", name=f"u_{_rep}")
            nc.vector.tensor_add(u_sb[:], t1[:], x2_ps[:, 0:NPIX])
            # softmax denominator -> normalized A
            s_ps = ps.tile([BL, NPIX], f32, tag="B", bufs=2, name=f"s_{_rep}")
            nc.tensor.matmul(s_ps[:], bmT_sb[:], a_u[:, 0:NPIX], start=True, stop=True)
            s_r = sp.tile([BL, NPIX], f32, tag="sr", name=f"sr_{_rep}")
            nc.vector.reciprocal(s_r[:], s_ps[:])
            rb_ps = ps.tile([128, NPIX], f32, tag="B", bufs=2, name=f"rb_{_rep}")
            nc.tensor.matmul(rb_ps[:], bm_sb[:], s_r[:], start=True, stop=True)
            a_t = ap_.tile([128, PPIX], f32, tag="at", name=f"at_{_rep}")
            nc.vector.tensor_mul(a_t[:, 0:NPIX], a_u[:, 0:NPIX], rb_ps[:])
            nc.vector.memset(a_t[:, NPIX:PPIX], 0.0)
            # Atil2 = 2*Atilde = A * (x2 - dist)
            at2 = ap_.tile([128, PPIX], f32, tag="at2", name=f"at2_{_rep}")
            nc.vector.tensor_mul(at2[:, 0:NPIX], u_sb[:], a_t[:, 0:NPIX])
            nc.vector.memset(at2[:, NPIX:PPIX], 0.0)

            if STAGE >= 2:
                # ============ Gram / patch norms ============
                n2_ps = ps.tile([128, 512], f32, tag="N", bufs=1, name=f"n2_{_rep}")
                nc.vector.memset(n2_ps[:], 1.0)  # unused lanes benign (sqrt/recip finite)
                yts = []
                for b in range(BL):
                    asl = a_t[32 * b:32 * b + 32, :]
                    a2sl = at2[32 * b:32 * b + 32, :]
                    yys1 = []
                    wsb = []
                    for q in range(2):
                        yy_ps = ps.tile([128, PPIX], f32, tag="D", bufs=3, name=f"yy_{_rep}_{b}_{q}")
                        nc.tensor.matmul(yy_ps[:], rg(yb(b, 128 * q, 128 * q + 128)),
                                         rg(yb(b, 0, PPIX)), start=True, stop=True)
                        s1_ps = ps.tile([128, PPIX], f32, tag="D", bufs=3, name=f"s1_{_rep}_{b}_{q}")
                        nc.tensor.matmul(s1_ps[:], rg(asl[:, 128 * q:128 * q + 128]),
                                         rg(asl[:]), start=True, stop=True,
                                         tile_position=(32 * b, 0))
                        w_ps = ps.tile([128, PPIX], f32, tag="D", bufs=3, name=f"w_{_rep}_{b}_{q}")
                        nc.tensor.matmul(w_ps[:], rg(a2sl[:, 128 * q:128 * q + 128]),
                                         rg(asl[:]), start=True, stop=False,
                                         tile_position=(32 * b, 0))
                        nc.tensor.matmul(w_ps[:], rg(asl[:, 128 * q:128 * q + 128]),
                                         rg(a2sl[:]), start=False, stop=True,
                                         tile_position=(32 * b, 0))
                        s1c = gp.tile([128, PPIX], f32, tag=f"s1c{q}", name=f"s1c_{_rep}_{b}_{q}")
                        nc.scalar.copy(s1c[:], s1_ps[:])
                        g1 = gp.tile([128, PPIX], f32, tag=f"g1{q}", name=f"g1_{_rep}_{b}_{q}")
                        nc.vector.tensor_mul(g1[:], s1c[:], yy_ps[:])
                        wc = gp.tile([128, PPIX], f32, tag=f"wc{q}", name=f"wc_{_rep}_{b}_{q}")
                        nc.vector.tensor_copy(wc[:], w_ps[:])
                        yys1.append(g1)
                        wsb.append(wc)
                    # T = G @ M^T  (two output row-chunks r)
                    for r in range(2):
                        t_ps = ps.tile([128, NP_], f32, tag="A", bufs=2, name=f"t_{_rep}_{b}_{r}")
                        for q in range(2):
                            nc.tensor.matmul(t_ps[:], yys1[q][:, 128 * r:128 * r + 128],
                                             mt_sb[:, q, :], start=(q == 0), stop=False)
                            nc.tensor.matmul(t_ps[:], wsb[q][:, 128 * r:128 * r + 128],
                                             mth_sb[:, q, :], start=False, stop=(q == 1))
                        prod = sp.tile([128, NP_], f32, tag="prod", name=f"prod_{_rep}_{b}_{r}")
                        nc.vector.tensor_mul(prod[:], mt_sb[:, r, :], t_ps[:])
                        nc.tensor.matmul(n2_ps[32 * b:32 * b + 1, 0:NP_], onec_sb[:], prod[:],
                                         start=(r == 0), stop=(r == 1),
                                         tile_position=(0, 32 * b))
                    # y^T tiles for the final aggregation
                    ytb = []
                    for q in range(2):
                        ytq_ps = ps.tile([128, 128], f32, tag="A", bufs=2, name=f"ytps_{_rep}_{b}_{q}")
                        nc.tensor.transpose(ytq_ps[:], yb(b, 128 * q, 128 * q + 128), ident_sb[:])
                        ytq = ytp.tile([128, 128], f32, tag="yt", name=f"yt_{_rep}_{b}_{q}")
                        if q == 0:
                            nc.scalar.copy(ytq[:], ytq_ps[:])
                        else:
                            nc.vector.tensor_copy(ytq[:], ytq_ps[:])
                        ytb.append(ytq)
                    yts.append(ytb)

            if STAGE >= 3:
                # ============ patch weights w ============
                nrm = sp.tile([128, NP_], f32, tag="nrm", name=f"nrm_{_rep}")
                nc.scalar.activation(nrm[:], n2_ps[:, 0:NP_], AF.Sqrt)
                invn = sp.tile([128, NP_], f32, tag="invn", name=f"invn_{_rep}")
                nc.vector.reciprocal(invn[:], nrm[:])
                it_ps = ps.tile([128, 128], f32, tag="N", bufs=1, name=f"itps_{_rep}")
                nc.tensor.transpose(it_ps[0:NP_, :], invn[:], ident_sb[:])
                invnT = sp.tile([NP_, 128], f32, tag="invnT", name=f"invnT_{_rep}")
                nc.scalar.copy(invnT[:], it_ps[0:NP_, :])
                w_ps = ps.tile([BL, PPIX], f32, tag="B", bufs=2, name=f"wps_{_rep}")
                invnT4 = invnT.rearrange("p (a b) -> p a b", b=32)[:, :, 0]
                nc.tensor.matmul(w_ps[:], invnT4, mmat_sb[:], start=True, stop=True)
                w_sb = sp.tile([BL, PPIX], f32, tag="wsb", name=f"wsb_{_rep}")
                nc.scalar.copy(w_sb[:], w_ps[:])
                wb_ps = ps.tile([128, PPIX], f32, tag="B", bufs=2, name=f"wb_{_rep}")
                nc.tensor.matmul(wb_ps[:], bm_sb[:], w_sb[:], start=True, stop=True)
                wa = ap_.tile([128, PPIX], f32, tag="wa", name=f"wa_{_rep}")
                nc.vector.tensor_mul(wa[:], a_t[:], wb_ps[:])
                v2 = sp.tile([128, 1], f32, tag="v2", name=f"v2_{_rep}")
                nc.vector.reduce_sum(v2[:], wa[:], axis=AX.X)
                v2n = sp.tile([128, 1], f32, tag="v2n", name=f"v2n_{_rep}")
                nc.vector.tensor_scalar_mul(v2n[:], v2[:], -1.0)

                # WA^T
                wat2 = []
                for q in range(2):
                    wt_ps = ps.tile([128, 128], f32, tag="D", bufs=3, name=f"watps_{_rep}_{q}")
                    nc.tensor.transpose(wt_ps[:], wa[:, 128 * q:128 * q + 128], ident_sb[:])
                    wq = gp.tile([128, 128], f32, tag=f"wat{q}", name=f"wat_{_rep}_{q}")
                    nc.vector.tensor_copy(wq[:], wt_ps[:])
                    wat2.append(wq)

                # ============ v = sum_pix w*A*(y-c) ============
                v1_ps = ps.tile([128, 128], f32, tag="A", bufs=2, name=f"v1_{_rep}")
                for b in range(BL):
                    for q in range(2):
                        nc.tensor.matmul(v1_ps[32 * b:32 * b + 32, :],
                                         wat2[q][:, 32 * b:32 * b + 32], yts[b][q][:],
                                         start=(q == 0), stop=(q == 1),
                                         tile_position=(0, 32 * b))
                v_sb = sp.tile([128, 128], f32, tag="vsb", name=f"v_{_rep}")
                nc.vector.scalar_tensor_tensor(out=v_sb[:], in0=ctile_sb[:], scalar=v2n[:, 0:1],
                                               in1=v1_ps[:], op0=OP.mult, op1=OP.add)
                vt_ps = ps.tile([128, 128], f32, tag="D", bufs=3, name=f"vtps_{_rep}")
                nc.tensor.transpose(vt_ps[:], v_sb[:], ident_sb[:])
                vt_sb = sp.tile([128, 128], f32, tag="vt", name=f"vt_{_rep}")
                nc.scalar.copy(vt_sb[:], vt_ps[:])
                if WRITE_V:
                    nc.gpsimd.dma_start(out=vout[:], in_=vt_sb[:])

            if STAGE >= 4:
                # ============ AllGather v ============
                v_loc = dp.tile([128, 128], f32, tag="vloc", name=f"vloc_{_rep}")
                v_gth = dp.tile([NCORES * 128, 128], f32, tag="vgth", name=f"vgth_{_rep}",
                                addr_space="Shared")
                nc.gpsimd.dma_start(out=v_loc[:], in_=vt_sb[:])
                nc.gpsimd.collective_compute(
                    "AllGather", OP.bypass, replica_groups=[list(range(NCORES))],
                    ins=[v_loc.opt()], outs=[v_gth.opt()])
                vall = yp.tile([128, NCORES, 128], f32, tag="vall", name=f"vall_{_rep}")
                nc.gpsimd.dma_start(out=vall[:], in_=v_gth.rearrange("(c d) n -> d c n", d=128))

                # ============ classifier (class shard) ============
                lg_ps = ps.tile([CSH, 32], f32, tag="A", bufs=2, name=f"lg_{_rep}")
                vk = vall.rearrange("d c (b k) -> d c b k", k=32)
                for j in range(K):
                    nc.tensor.matmul(lg_ps[:], clsw_sb[:, j, :], vk[:, :, :, j],
                                     start=(j == 0), stop=(j == K - 1))
                lg_sb = sp.tile([CSH, 32], f32, tag="lg", name=f"lgsb_{_rep}")
                nc.scalar.activation(lg_sb[:], lg_ps[:], AF.Identity, bias=clsb_sb[:, 0:1])
                nc.gpsimd.dma_start(out=logits[:], in_=lg_sb[:])

    nc.compile()
    return nc


def build_cls_nc():
    """Classifier-only kernel: replicated v_all input, per-core class shard."""
    import concourse.tile as tile
    from concourse import bacc, mybir
    f32 = mybir.dt.float32
    AF = mybir.ActivationFunctionType
    nc = bacc.Bacc("TRN2", target_bir_lowering=False, debug=False, num_devices=NCORES)
    vin = nc.dram_tensor("vin", [NCORES * 128, 128], f32, kind="ExternalInput").ap()
    clsw = nc.dram_tensor("clsw", [K, 128, CSH], f32, kind="ExternalInput").ap()
    clsb = nc.dram_tensor("clsb", [CSH, 1], f32, kind="ExternalInput").ap()
    logits = nc.dram_tensor("logits", [CSH, B], f32, kind="ExternalOutput").ap()
    with tile.TileContext(nc) as tc, ExitStack() as ctx:
        cp = ctx.enter_context(tc.tile_pool(name="consts", bufs=1))
        ps = ctx.enter_context(tc.tile_pool(name="ps", bufs=1, space="PSUM"))
        clsw_sb = cp.tile([128, K, CSH], f32)
        nc.gpsimd.dma_start(out=clsw_sb[:], in_=clsw.rearrange("k p n -> p k n"))
        clsb_sb = cp.tile([CSH, 1], f32)
        nc.gpsimd.dma_start(out=clsb_sb[:], in_=clsb[:])
        vall = cp.tile([128, NCORES, 128], f32)
        nc.gpsimd.dma_start(out=vall[:], in_=vin.rearrange("(c d) n -> d c n", d=128))
        lg_ps = ps.tile([CSH, 32], f32, tag="A", bufs=1)
        vk = vall.rearrange("d c (b k) -> d c b k", k=32)
        for j in range(K):
            nc.tensor.matmul(lg_ps[:], clsw_sb[:, j, :], vk[:, :, :, j],
                             start=(j == 0), stop=(j == K - 1))
        lg_sb = cp.tile([CSH, 32], f32)
        nc.scalar.activation(lg_sb[:], lg_ps[:], AF.Identity, bias=clsb_sb[:, 0:1])
        nc.gpsimd.dma_start(out=logits[:], in_=lg_sb[:])
    nc.compile()
    return nc


# ----------------------------------------------------------------- host side
def make_inputs(x, conv_w, conv_b, bn_gamma, bn_beta, bn_mean, bn_var,
                codewords, scale, cls_w, cls_b):
    f = np.float32
    inv = (bn_gamma / np.sqrt(bn_var + BN_EPS)).astype(np.float64)
    wc = (conv_w.astype(np.float64) * inv[:, None]).astype(f)          # [D, CIN]
    biasd = ((conv_b - bn_mean).astype(np.float64) * inv + bn_beta).astype(f)
    c2 = (codewords.astype(np.float64) ** 2).sum(1).astype(f)          # [K]
    cw2 = (-2.0 * scale.astype(np.float64)[:, None]
           * codewords.astype(np.float64)).astype(f)                   # [K, D]

    consts = {}
    consts["wcT"] = np.ascontiguousarray(wc.T.reshape(NCH, 128, 128))
    consts["biasd"] = biasd.reshape(128, 1)
    consts["cw2T"] = np.ascontiguousarray(cw2.T)                       # [128, K]
    consts["scbc"] = np.broadcast_to(scale.astype(f), (128, K)).copy()
    consts["ones32"] = np.ones((128, K), f)
    consts["ebias"] = np.tile((scale * c2).astype(f), BL).reshape(128, 1)
    consts["negis"] = np.tile((-1.0 / scale).astype(f), BL).reshape(128, 1)
    consts["negc2"] = np.tile((-c2).astype(f), BL).reshape(128, 1)
    consts["ctile"] = np.tile(codewords.astype(f), (BL, 1))            # [128, 128]
    bmT = np.zeros((128, BL), f)
    for b in range(BL):
        bmT[32 * b:32 * b + 32, b] = 1.0
    consts["bmT"] = bmT
    consts["bm"] = np.ascontiguousarray(bmT.T)
    consts["onec"] = np.ones((128, 1), f)
    m = np.zeros((NP_, PPIX), f)
    for r in range(PH):
        for c in range(PW):
            for di in range(WIN):
                for dj in range(WIN):
                    m[r * PW + c, (r + di) * WW + (c + dj)] = 1.0
    consts["mmat"] = m
    mt = np.ascontiguousarray(m.T.reshape(2, 128, NP_))
    consts["mtc"] = mt
    consts["mth"] = (-0.5 * mt).astype(f)
    consts["ident"] = np.eye(128, dtype=f)

    cls_wT = np.ascontiguousarray(cls_w.T)                             # [4096, 1000]
    in_maps = []
    for c in range(NCORES):
        im = dict(consts)
        xs = np.zeros((CIN, BL * PPIX), f)
        for b in range(BL):
            xs[:, PPIX * b:PPIX * b + NPIX] = x[BL * c + b].reshape(CIN, NPIX)
        im["xin"] = xs.reshape(NCH, 128, BL * PPIX)
        sh = cls_wT[:, CSH * c:CSH * (c + 1)]                          # [4096, 125]
        im["clsw"] = np.ascontiguousarray(sh.reshape(K, 128, CSH))
        im["clsb"] = np.ascontiguousarray(cls_b[CSH * c:CSH * (c + 1)].reshape(CSH, 1))
        in_maps.append(im)
    return in_maps


def assemble(results):
    return np.concatenate([results[c]["logits"].T for c in range(NCORES)], axis=1)


USE_COLLECTIVE = False


def kernel(**inputs):
    global STAGE, WRITE_V
    inputs = {k: np.asarray(v) for k, v in inputs.items()}
    from concourse.bass_utils import run_bass_kernel_spmd
    in_maps = make_inputs(**inputs)
    if USE_COLLECTIVE:
        key = ("full", F32R_CONV, F32R_GRAM)
        if key not in _NC_CACHE:
            STAGE, WRITE_V = 4, False
            _NC_CACHE[key] = build_nc(1)
        res = run_bass_kernel_spmd(_NC_CACHE[key], in_maps, list(range(NCORES)))
        return assemble(res.results)
    key = ("v", F32R_CONV, F32R_GRAM)
    if key not in _NC_CACHE:
        STAGE, WRITE_V = 3, True
        _NC_CACHE[key] = build_nc(1)
        _NC_CACHE["cls"] = build_cls_nc()
    res1 = run_bass_kernel_spmd(_NC_CACHE[key], in_maps, list(range(NCORES)))
    vall = np.concatenate([res1.results[c]["vout"] for c in range(NCORES)], axis=0)
    in_maps2 = [{"vin": vall, "clsw": in_maps[c]["clsw"], "clsb": in_maps[c]["clsb"]}
                for c in range(NCORES)]
    res2 = run_bass_kernel_spmd(_NC_CACHE["cls"], in_maps2, list(range(NCORES)))
    return assemble(res2.results)

